# revision 19
# baseline (speedup 1.0000x reference)
"""Trainium2 Bass kernel for nn_MultiHeadAttention_75213467287764.

Multi-head attention (B=2, L=1024, D=1024, H=16, Dh=64) with x-LN, fused
qkv, q/k-LN, rotary, SDPA, and a geometric attention bias, sharded over
8 NeuronCores: 2 heads per core, both batches (head/tensor parallel).

Design notes (activations kept in transposed [channel, token] layout):
  - x-LN is distributed through the qkv matmul: qkv_true = r[t]*(W'x
    - mu[t]*(W'1) + sigma[t]*cb), W' = wqkv*ln_w, cb = W'@ln_b, applied
    as two augmented accumulation rows. The per-token scale r[t] cancels
    inside q/k-LN; for v it is folded into the v transpose-evacuation.
  - q/k-LN stats need full 1024-channel sums: each core computes partial
    (sum, sumsq) over its 128 channels via gpsimd partition_all_reduce;
    one 32 KB AllReduce combines cores. Exact eps handling:
    w = 1/sqrt(var_corr + eps*sigma_x^2).
  - rotary via a constant block-swap matmul J: rot(y) = cos*y + sin*(J@y).
    q_ln_w / k_ln_w applied per-partition; w_q[t] via gpsimd
    partition_broadcast + multiply; w_k[t] folds into the softmax exp scale.
  - scores computed transposed S_T[k,q]; exp on ScalarE with per-partition
    scale; softmax denominator via all-ones columns inside the AV matmul
    ([1|v_h0|1|v_h1] lhsT), divided out after.
  - geo bias collapses to rank-12 (mask is all ones, so g_scores is
    linear): W_kb = k_nat^T @ (bm/len); tmp2 = W_kb^T @ q_rot; expand,
    one elementwise multiply with expanded base, then the edge_w matmul.
  - out_proj row-parallel: each core emits a full-shape partial out_T;
    the host sums the 8 partials and transposes back.
"""
import numpy as np

B, L, D, H, Dh = 2, 1024, 1024, 16, 64
T = B * L
N_CORES = 8
EPS = 1e-5
P = 128
NT = T // 512          # 4 column chunks of 512 tokens
NI = D // P            # 8 contraction chunks
NC16 = T // P          # 16 token chunks of 128

TRACE = False          # set by test harness for profiling
DEBUG = False          # add intermediate-dump outputs
LAST_EXEC_NS = None

_CACHED = {}


def _host_prep(inputs):
    x = np.ascontiguousarray(np.asarray(inputs["x"], np.float32))
    mask = np.asarray(inputs["attention_mask"])
    blocks = np.asarray(inputs["blocks"], np.float32)
    ln_w = np.asarray(inputs["ln_w"], np.float32)
    ln_b = np.asarray(inputs["ln_b"], np.float32)
    wqkv = np.asarray(inputs["wqkv"], np.float32)
    q_ln_w = np.asarray(inputs["q_ln_w"], np.float32)
    k_ln_w = np.asarray(inputs["k_ln_w"], np.float32)
    out_w = np.asarray(inputs["out_w"], np.float32)
    edge_w = np.asarray(inputs["edge_w"], np.float32)
    position = np.asarray(inputs["position"], np.float32)

    assert mask.all(), "kernel assumes an all-ones attention mask"
    length = mask.astype(np.float32).sum(-1)            # (B, L)

    x_T = np.ascontiguousarray(x.reshape(T, D).T)       # (D, T)

    Wp = wqkv * ln_w[None, :]
    sp = Wp.sum(1)
    cb = Wp @ ln_b

    inv_freq = 1.0 / (10000.0 ** (np.arange(0, Dh, 2, dtype=np.float32) / Dh))
    freqs = position[..., None] * inv_freq
    emb = np.concatenate([freqs, freqs], -1)            # (B, L, 64)
    cosD = np.cos(emb).reshape(T, Dh).T                 # (64, T)
    sinD = np.sin(emb).reshape(T, Dh).T
    cos2 = np.ascontiguousarray(np.vstack([cosD, cosD]).astype(np.float32))
    sin2 = np.ascontiguousarray(np.vstack([sinD, sinD]).astype(np.float32))

    J = np.zeros((Dh, Dh), np.float32)
    J[np.arange(32), np.arange(32) + 32] = -1.0
    J[np.arange(32) + 32, np.arange(32)] = 1.0
    J2 = np.zeros((P, P), np.float32)
    J2[:64, :64] = J
    J2[64:, 64:] = J
    J2T = np.ascontiguousarray(J2.T)

    M = blocks.mean(axis=-2, keepdims=True)
    bm = blocks - M
    eps_f = np.finfo(np.float32).eps
    base = bm / (np.linalg.norm(bm, axis=-1, keepdims=True) + eps_f)
    bm_sc = (bm / length[:, :, None, None]).reshape(T, 12)

    base_ex = np.zeros((B, 48, L), np.float32)
    rep = np.zeros((12, 48), np.float32)
    ew_lhsT = np.zeros((48, 64), np.float32)
    for e in range(4):
        for c in range(4):
            for xx in range(3):
                idx = e * 12 + c * 3 + xx
                base_ex[:, idx, :] = base[:, :, e, xx]
                rep[c * 3 + xx, idx] = 1.0
                ew_lhsT[idx, :] = edge_w[:, e * 4 + c]

    # bm_sc chunk-major: col block j (12 wide) = token chunk j
    bm_sc_cm = np.ascontiguousarray(
        bm_sc.reshape(NC16, P, 12).transpose(1, 0, 2).reshape(P, NC16 * 12))

    per_core = []
    for c in range(N_CORES):
        r0 = P * c
        rows = np.r_[r0:r0 + P, D + r0:D + r0 + P, 2 * D + r0:2 * D + r0 + P]
        W_own_T = np.ascontiguousarray(Wp[rows].T)       # (1024, 384)
        per_core.append(dict(
            w_own_T=W_own_T,
            aug_mu=np.ascontiguousarray(-sp[rows][None, :]),
            aug_cb=np.ascontiguousarray(cb[rows][None, :]),
            qlw_col=np.ascontiguousarray(q_ln_w[r0:r0 + P, None]),
            klw_col=np.ascontiguousarray(k_ln_w[r0:r0 + P, None]),
            outw_T=np.ascontiguousarray(out_w[:, r0:r0 + P].T),  # (128, 1024)
        ))

    shared = dict(
        x_T=x_T, cos2=cos2, sin2=sin2, J2T=J2T,
        bm_sc_cm=bm_sc_cm, base_ex=np.ascontiguousarray(base_ex),
        rep_lhsT=rep, ew_lhsT=ew_lhsT,
        ones_over_D=np.full((P, 1), 1.0 / D, np.float32),
        ones_row=np.ones((1, P), np.float32),
    )
    return shared, per_core


def _build_nc():
    import concourse.bacc as bacc
    import concourse.tile as tile
    import concourse.mybir as mybir
    from concourse import masks, bass_isa

    f32 = mybir.dt.float32
    f32r = mybir.dt.float32r
    AF = mybir.ActivationFunctionType
    OP = mybir.AluOpType

    nc = bacc.Bacc("TRN2", target_bir_lowering=False, debug=False,
                   enable_asserts=False, num_devices=N_CORES)

    d_xT = nc.dram_tensor("x_T", [D, T], f32r, kind="ExternalInput")
    d_w = nc.dram_tensor("w_own_T", [D, 384], f32r, kind="ExternalInput")
    d_augmu = nc.dram_tensor("aug_mu", [1, 384], f32r, kind="ExternalInput")
    d_augcb = nc.dram_tensor("aug_cb", [1, 384], f32r, kind="ExternalInput")
    d_qlw = nc.dram_tensor("qlw_col", [P, 1], f32, kind="ExternalInput")
    d_klw = nc.dram_tensor("klw_col", [P, 1], f32, kind="ExternalInput")
    d_cos2 = nc.dram_tensor("cos2", [P, T], f32, kind="ExternalInput")
    d_sin2 = nc.dram_tensor("sin2", [P, T], f32, kind="ExternalInput")
    d_J2T = nc.dram_tensor("J2T", [P, P], f32r, kind="ExternalInput")
    d_outw = nc.dram_tensor("outw_T", [P, D], f32r, kind="ExternalInput")
    d_bmsc = nc.dram_tensor("bm_sc_cm", [P, NC16 * 12], f32r, kind="ExternalInput")
    d_base = nc.dram_tensor("base_ex", [B, 48, L], f32, kind="ExternalInput")
    d_rep = nc.dram_tensor("rep_lhsT", [12, 48], f32r, kind="ExternalInput")
    d_ew = nc.dram_tensor("ew_lhsT", [48, 64], f32r, kind="ExternalInput")
    d_oD = nc.dram_tensor("ones_over_D", [P, 1], f32r, kind="ExternalInput")
    d_ones = nc.dram_tensor("ones_row", [1, P], f32r, kind="ExternalInput")
    d_out = nc.dram_tensor("out_T", [D, T], f32, kind="ExternalOutput")
    if DEBUG:
        d_dbg_corr = nc.dram_tensor("dbg_corr", [384, T], f32, kind="ExternalOutput")
        d_dbg_rot = nc.dram_tensor("dbg_rot", [256, T], f32, kind="ExternalOutput")
        d_dbg_st = nc.dram_tensor("dbg_st", [8, T], f32, kind="ExternalOutput")
        d_dbg_ctx = nc.dram_tensor("dbg_ctx", [256, L], f32, kind="ExternalOutput")
        d_dbg_cols = nc.dram_tensor("dbg_cols", [P, 3 * NC16], f32, kind="ExternalOutput")
        d_dbg_att = nc.dram_tensor("dbg_att", [4 * P, L], f32, kind="ExternalOutput")
        d_dbg_v1 = nc.dram_tensor("dbg_v1", [P, 260], f32, kind="ExternalOutput")

    with tile.TileContext(nc) as tc:
        with tc.tile_pool(name="cst", bufs=1) as cst, \
             tc.tile_pool(name="dram", bufs=1, space="DRAM") as dram:

            # ---------------- persistent constants / buffers ------------
            idn = cst.tile([P, P], f32, tag="idn")
            masks.make_identity(nc, idn[:])
            ws = []
            for i in range(NI):
                wt_ = cst.tile([P, 384], f32r, tag=f"ws{i}", name=f"ws{i}")
                nc.sync.dma_start(wt_[:], d_w[i * P:(i + 1) * P, :])
                ws.append(wt_)
            augmu = cst.tile([1, 384], f32r, tag="augmu")
            nc.sync.dma_start(augmu[:], d_augmu[:])
            augcb = cst.tile([1, 384], f32r, tag="augcb")
            nc.sync.dma_start(augcb[:], d_augcb[:])
            qlw = cst.tile([P, 1], f32, tag="qlw")
            nc.sync.dma_start(qlw[:], d_qlw[:])
            klw = cst.tile([P, 1], f32, tag="klw")
            nc.sync.dma_start(klw[:], d_klw[:])
            J2T = cst.tile([P, P], f32r, tag="J2T")
            nc.sync.dma_start(J2T[:], d_J2T[:])
            outw = cst.tile([P, D], f32r, tag="outw")
            nc.sync.dma_start(outw[:], d_outw[:])
            bmsc = cst.tile([P, NC16 * 12], f32r, tag="bmsc")
            nc.sync.dma_start(bmsc[:], d_bmsc[:])
            base_sb = []
            for b in range(B):
                bt_ = cst.tile([48, L], f32, tag=f"base{b}", name=f"base{b}")
                nc.sync.dma_start(bt_[:], d_base[b])
                base_sb.append(bt_)
            rep_t = cst.tile([12, 48], f32r, tag="rep")
            nc.sync.dma_start(rep_t[:], d_rep[:])
            ew_t = cst.tile([48, 64], f32r, tag="ew")
            nc.sync.dma_start(ew_t[:], d_ew[:])
            oD = cst.tile([P, 1], f32r, tag="oD")
            nc.sync.dma_start(oD[:], d_oD[:])
            ones_r = cst.tile([1, P], f32r, tag="ones_r")
            nc.sync.dma_start(ones_r[:], d_ones[:])
            eps_t = cst.tile([P, 1], f32, tag="eps_t")
            nc.vector.memset(eps_t[:], EPS)

            corr = [cst.tile([P, T], f32r, tag=f"corr{j}", name=f"corr{j}")
                    for j in range(3)]
            corr_q, corr_k, corr_v = corr
            qrot = cst.tile([P, T], f32r, tag="qrot")
            krot = cst.tile([P, T], f32r, tag="krot")
            ctxcat = [cst.tile([P, L], f32r, tag=f"ctxcat{b}", name=f"ctxcat{b}")
                      for b in range(B)]
            s2eps = cst.tile([1, T], f32, tag="s2eps")
            r_col = cst.tile([P, NC16], f32, tag="r_col")
            wk_col = cst.tile([P, NC16], f32, tag="wk_col")
            sc_col = cst.tile([P, NC16], f32, tag="sc_col")
            bounce = dram.tile([8, T], f32)
            ar_in = dram.tile([4, T], f32)
            ar_out = dram.tile([4, T], f32)

            # ================ phase A: x load, stats, qkv ================
            with tc.tile_pool(name="pha", bufs=1) as pha:
                xs = []
                for i in range(NI):
                    xt_ = pha.tile([P, T], f32r, tag=f"xs{i}", name=f"xs{i}")
                    nc.sync.dma_start(xt_[:], d_xT[i * P:(i + 1) * P, :])
                    xs.append(xt_)

                mu_r = pha.tile([1, T], f32r, tag="mu_r")
                sig_r = pha.tile([1, T], f32r, tag="sig_r")
                musq = pha.tile([1, T], f32, tag="musq")
                var_x = pha.tile([1, T], f32, tag="var_x")
                r_row = pha.tile([1, T], f32, tag="r_row")
                with tc.tile_pool(name="psA1", bufs=1, space="PSUM") as psA1:
                    ps_sq = psA1.tile([1, T], f32, tag="ps_sq", bufs=1)
                    for i in range(NI):
                        for half in range(2):
                            hsl = slice(half * 1024, (half + 1) * 1024)
                            x2t = pha.tile([P, 1024], f32r, tag="x2t", bufs=2)
                            nc.scalar.activation(x2t[:], xs[i][:, hsl],
                                                 AF.Square)
                            for n in range(2):
                                sl = slice(half * 1024 + n * 512,
                                           half * 1024 + (n + 1) * 512)
                                nc.tensor.matmul(
                                    ps_sq[:, sl], oD[:],
                                    x2t[:, n * 512:(n + 1) * 512],
                                    start=(i == 0), stop=(i == NI - 1))
                    ps_mu = psA1.tile([1, T], f32, tag="ps_mu", bufs=1)
                    for n in range(NT):
                        sl = slice(n * 512, (n + 1) * 512)
                        for i in range(NI):
                            nc.tensor.matmul(ps_mu[:, sl], oD[:], xs[i][:, sl],
                                             start=(i == 0), stop=(i == NI - 1))

                    # massage: base-0 row tiles (same-start-partition rule)
                    nc.vector.tensor_copy(mu_r[:], ps_mu[:])
                    nc.scalar.activation(musq[:], ps_mu[:], AF.Square)
                    nc.vector.tensor_tensor(var_x[:], ps_sq[:], musq[:],
                                            OP.subtract)
                    nc.scalar.activation(sig_r[:], var_x[:], AF.Sqrt,
                                         bias=eps_t[0:1, :])
                    nc.vector.reciprocal(r_row[:], sig_r[:].bitcast(f32))
                    nc.vector.tensor_scalar(s2eps[:], var_x[:], EPS,
                                            EPS * EPS, OP.mult, OP.add)
                    # r in column-major [128,16] via DRAM bounce
                    nc.sync.dma_start(bounce[2:3, :], r_row[:])
                    nc.sync.dma_start(
                        r_col[:], bounce[2, :].rearrange("(a b) -> b a", b=P))

                # qkv matmuls + aug rows
                with tc.tile_pool(name="psA2", bufs=1, space="PSUM") as psA2:
                    for jc in range(3):
                        jsl = slice(jc * P, (jc + 1) * P)
                        for n in range(NT):
                            sl = slice(n * 512, (n + 1) * 512)
                            pst = psA2.tile([P, 512], f32, tag="qkv_ps", bufs=4)
                            for i in range(NI):
                                nc.tensor.matmul(pst[:], ws[i][:, jsl],
                                                 xs[i][:, sl],
                                                 start=(i == 0), stop=False)
                            nc.tensor.matmul(pst[:], augmu[:, jsl],
                                             mu_r[:, sl],
                                             start=False, stop=False)
                            nc.tensor.matmul(pst[:], augcb[:, jsl],
                                             sig_r[:, sl],
                                             start=False, stop=True)
                            if n % 2 == 0:
                                nc.vector.tensor_copy(corr[jc][:, sl], pst[:])
                            else:
                                nc.scalar.copy(corr[jc][:, sl], pst[:])

            # ================ phase B: qk stats, allreduce, rotary =======
            with tc.tile_pool(name="phb", bufs=1) as phb:
                # ar rows: 0 Sq | 1 Sk | 2 Sq2 | 3 Sk2
                par = phb.tile([P, T], f32, tag="par", bufs=1)
                nc.gpsimd.partition_all_reduce(
                    par[:], corr_q[:].bitcast(f32), channels=P,
                    reduce_op=bass_isa.ReduceOp.add)
                nc.gpsimd.dma_start(ar_in[0:1, :], par[0:1, :])
                par2 = phb.tile([P, T], f32, tag="par", bufs=1)
                nc.gpsimd.partition_all_reduce(
                    par2[:], corr_k[:].bitcast(f32), channels=P,
                    reduce_op=bass_isa.ReduceOp.add)
                nc.gpsimd.dma_start(ar_in[1:2, :], par2[0:1, :])
                for r_i, src in ((2, corr_q), (3, corr_k)):
                    sqt = phb.tile([P, T], f32r, tag="sqt", bufs=2)
                    nc.scalar.activation(sqt[:], src[:], AF.Square)
                    par3 = phb.tile([P, T], f32, tag="par", bufs=1)
                    nc.gpsimd.partition_all_reduce(
                        par3[:], sqt[:].bitcast(f32), channels=P,
                        reduce_op=bass_isa.ReduceOp.add)
                    nc.gpsimd.dma_start(ar_in[r_i:r_i + 1, :], par3[0:1, :])

                nc.gpsimd.collective_compute(
                    "AllReduce", mybir.AluOpType.add,
                    replica_groups=[list(range(N_CORES))],
                    ins=[ar_in[:].opt()], outs=[ar_out[:].opt()])
                cos2 = phb.tile([P, T], f32, tag="cos2")
                nc.sync.dma_start(cos2[:], d_cos2[:])
                sin2 = phb.tile([P, T], f32, tag="sin2")
                nc.sync.dma_start(sin2[:], d_sin2[:])

                # base-0 pair tiles ([2,T]: row0=q, row1=k)
                mu_qk = phb.tile([2, T], f32, tag="mu_qk")
                w_qk = phb.tile([2, T], f32, tag="w_qk")
                mu_k_row = phb.tile([1, T], f32, tag="mu_k_row")
                sum_qk = phb.tile([2, T], f32, tag="qkrow", bufs=3,
                                  name="sum_qk")
                nc.sync.dma_start(sum_qk[:], ar_out[0:2, :])
                sum2_qk = phb.tile([2, T], f32, tag="qkrow", bufs=3,
                                   name="sum2_qk")
                nc.sync.dma_start(sum2_qk[:], ar_out[2:4, :])
                nc.vector.tensor_scalar(mu_qk[:], sum_qk[:], 1.0 / D, None,
                                        OP.mult)
                ex2 = phb.tile([2, T], f32, tag="qkrow", bufs=3, name="ex2")
                nc.vector.tensor_scalar(ex2[:], sum2_qk[:], 1.0 / D, None,
                                        OP.mult)
                m2q = phb.tile([2, T], f32, tag="qkrow", bufs=3, name="m2q")
                nc.scalar.activation(m2q[:], mu_qk[:], AF.Square)
                vr = phb.tile([2, T], f32, tag="qkrow", bufs=3, name="vr")
                nc.vector.tensor_tensor(vr[:], ex2[:], m2q[:], OP.subtract)
                s2e2 = phb.tile([2, T], f32, tag="qkrow", bufs=3, name="s2e2")
                nc.sync.dma_start(s2e2[0:1, :], s2eps[:])
                nc.sync.dma_start(s2e2[1:2, :], s2eps[:])
                ag = phb.tile([2, T], f32, tag="qkrow", bufs=3, name="ag")
                nc.vector.tensor_tensor(ag[:], vr[:], s2e2[:], OP.add)
                sq2 = phb.tile([2, T], f32, tag="qkrow", bufs=3, name="sq2")
                nc.scalar.activation(sq2[:], ag[:], AF.Sqrt)
                nc.vector.reciprocal(w_qk[:], sq2[:])
                # mu_k to its own base-0 home for partition_broadcast
                nc.sync.dma_start(mu_k_row[:], mu_qk[1:2, :])

                # wk column form + exp scale
                nc.sync.dma_start(bounce[4:5, :], w_qk[1:2, :])
                nc.sync.dma_start(
                    wk_col[:], bounce[4, :].rearrange("(a b) -> b a", b=P))
                nc.vector.tensor_scalar(sc_col[:], wk_col[:],
                                        1.0 / np.sqrt(Dh), None, OP.mult)

                # ---- LN apply + rotary (q then k) ----
                with tc.tile_pool(name="psB", bufs=1, space="PSUM") as psB:
                    def apply_rot(src, lw_col, mu_row, w_row, out_t, nm):
                        bmu = phb.tile([P, T], f32, tag="rt", bufs=3,
                                       name=f"bmu_{nm}")
                        nc.gpsimd.partition_broadcast(bmu[:], mu_row,
                                                      channels=P)
                        t1 = phb.tile([P, T], f32, tag="rt", bufs=3,
                                      name=f"t1_{nm}")
                        nc.vector.tensor_tensor(t1[:], src[:], bmu[:],
                                                OP.subtract)
                        t2 = phb.tile([P, T], f32r, tag="rt", bufs=3,
                                      name=f"t2_{nm}")
                        nc.vector.tensor_scalar(t2[:], t1[:], lw_col[:], None,
                                                OP.mult)
                        jp = psB.tile([P, T], f32, tag="j_ps", bufs=2)
                        for n in range(NT):
                            sl = slice(n * 512, (n + 1) * 512)
                            nc.tensor.matmul(jp[:, sl], J2T[:], t2[:, sl],
                                             start=True, stop=True)
                        m1 = phb.tile([P, T], f32, tag="rt", bufs=3,
                                      name=f"m1_{nm}")
                        nc.gpsimd.tensor_tensor(m1[:], t2[:], cos2[:], OP.mult)
                        m2 = phb.tile([P, T], f32, tag="rt", bufs=3,
                                      name=f"m2_{nm}")
                        nc.vector.tensor_tensor(m2[:], jp[:], sin2[:], OP.mult)
                        if w_row is None:
                            nc.vector.tensor_tensor(out_t[:], m1[:], m2[:],
                                                    OP.add)
                        else:
                            s12 = phb.tile([P, T], f32, tag="rt", bufs=3,
                                           name=f"s12_{nm}")
                            nc.vector.tensor_tensor(s12[:], m1[:], m2[:],
                                                    OP.add)
                            bw = phb.tile([P, T], f32, tag="rt", bufs=3,
                                          name=f"bw_{nm}")
                            nc.gpsimd.partition_broadcast(bw[:], w_row,
                                                          channels=P)
                            nc.vector.tensor_tensor(out_t[:], s12[:], bw[:],
                                                    OP.mult)

                    apply_rot(corr_q, qlw, mu_qk[0:1, :], w_qk[0:1, :],
                              qrot, "q")
                    apply_rot(corr_k, klw, mu_k_row[:], None, krot, "k")

            if DEBUG:
                for j in range(3):
                    nc.sync.dma_start(d_dbg_corr[j * P:(j + 1) * P, :],
                                      corr[j][:].bitcast(f32))
                nc.sync.dma_start(d_dbg_rot[0:P, :], qrot[:].bitcast(f32))
                nc.sync.dma_start(d_dbg_rot[P:2 * P, :], krot[:].bitcast(f32))
                nc.sync.dma_start(d_dbg_st[0:1, :], mu_r[:].bitcast(f32))
                nc.sync.dma_start(d_dbg_st[1:2, :], sig_r[:].bitcast(f32))
                nc.sync.dma_start(d_dbg_st[2:4, :], mu_qk[:])
                nc.sync.dma_start(d_dbg_st[4:6, :], w_qk[:])
                nc.sync.dma_start(d_dbg_st[6:7, :], s2eps[:])
                nc.sync.dma_start(d_dbg_st[7:8, :], mu_k_row[:])
                nc.sync.dma_start(d_dbg_cols[:, 0:NC16], r_col[:])
                nc.sync.dma_start(d_dbg_cols[:, NC16:2 * NC16], wk_col[:])
                nc.sync.dma_start(d_dbg_cols[:, 2 * NC16:3 * NC16], sc_col[:])

            # ================ phase C: transposes, attention, out ========
            with tc.tile_pool(name="phc", bufs=1) as phc:
                knat, v1 = [], []
                with tc.tile_pool(name="psC1", bufs=1, space="PSUM") as psC1:
                    for tch in range(NC16):
                        sl = slice(tch * P, (tch + 1) * P)
                        pt = psC1.tile([P, P], f32, tag="tr_ps", bufs=4)
                        nc.tensor.transpose(pt[:], krot[:, sl].bitcast(f32),
                                            idn[:])
                        kn = phc.tile([P, P], f32r, tag="knat", bufs=NC16,
                                      name=f"knat{tch}")
                        nc.vector.tensor_scalar(kn[:], pt[:],
                                                wk_col[:, tch:tch + 1], None,
                                                OP.mult)
                        knat.append(kn)
                        pv = psC1.tile([P, P], f32, tag="tr_ps", bufs=4)
                        nc.tensor.transpose(pv[:], corr_v[:, sl].bitcast(f32),
                                            idn[:])
                        vt = phc.tile([P, 130], f32r, tag="v1", bufs=NC16,
                                      name=f"v1_{tch}")
                        nc.sync.dma_start(
                            vt[:, 64:65],
                            d_ones[0:1, :].rearrange("a b -> b a"))
                        nc.sync.dma_start(
                            vt[:, 129:130],
                            d_ones[0:1, :].rearrange("a b -> b a"))
                        nc.vector.tensor_scalar(vt[:, 0:64], pv[:, 0:64],
                                                r_col[:, tch:tch + 1], None,
                                                OP.mult)
                        nc.vector.tensor_scalar(vt[:, 65:129], pv[:, 64:128],
                                                r_col[:, tch:tch + 1], None,
                                                OP.mult)
                        if DEBUG and tch == 0:
                            nc.sync.dma_start(d_dbg_v1[:, 0:130],
                                              vt[:].bitcast(f32))
                            nc.sync.dma_start(d_dbg_v1[:, 130:258],
                                              kn[:].bitcast(f32))
                        v1.append(vt)

                with tc.tile_pool(name="psC2", bufs=1, space="PSUM") as psC2:
                    for b in range(B):
                        for h in range(2):
                            hs = slice(h * 64, (h + 1) * 64)
                            vs = slice(h * 65, (h + 1) * 65)
                            ctx_ps = psC2.tile([65, L], f32, tag="ctx_ps",
                                               bufs=1)
                            wkb_ps = psC2.tile([64, 12], f32, tag="wkb_ps",
                                               bufs=1)
                            for kc in range(8):
                                tch = b * 8 + kc
                                ksl = slice(tch * P, (tch + 1) * P)
                                sps = psC2.tile([P, L], f32, tag="s_ps",
                                                bufs=1)
                                for n in range(2):
                                    qsl = slice(b * L + n * 512,
                                                b * L + (n + 1) * 512)
                                    nc.tensor.matmul(
                                        sps[:, n * 512:(n + 1) * 512],
                                        krot[hs, ksl], qrot[hs, qsl],
                                        start=True, stop=True)
                                pexp = phc.tile([P, L], f32r, tag="pexp",
                                                bufs=2)
                                nc.scalar.activation(
                                    pexp[:], sps[:], AF.Exp,
                                    scale=sc_col[:, tch:tch + 1])
                                if DEBUG and b == 0 and h == 0 and kc == 0:
                                    nc.sync.dma_start(
                                        d_dbg_att[3 * P:4 * P, :],
                                        pexp[:].bitcast(f32))
                                for n in range(2):
                                    nc.tensor.matmul(
                                        ctx_ps[:, n * 512:(n + 1) * 512],
                                        v1[tch][:, vs],
                                        pexp[:, n * 512:(n + 1) * 512],
                                        start=(kc == 0), stop=(kc == 7))
                                nc.tensor.matmul(
                                    wkb_ps[:], knat[tch][:, hs],
                                    bmsc[:, tch * 12:(tch + 1) * 12],
                                    start=(kc == 0), stop=(kc == 7))
                            # geo tail
                            wkb = phc.tile([P, 12], f32r, tag="wkb", bufs=2)
                            if h == 0:
                                nc.vector.tensor_copy(wkb[0:64, :], wkb_ps[:])
                            else:
                                wkb0 = phc.tile([64, 12], f32r, tag="wkb0",
                                                bufs=2)
                                nc.vector.tensor_copy(wkb0[:], wkb_ps[:])
                                nc.sync.dma_start(wkb[64:128, :], wkb0[:])
                            gps = psC2.tile([64, L], f32, tag="geo_ps", bufs=1,
                                            name="t2ps")
                            for n in range(2):
                                qsl = slice(b * L + n * 512,
                                            b * L + (n + 1) * 512)
                                nc.tensor.matmul(
                                    gps[0:12, n * 512:(n + 1) * 512],
                                    wkb[hs, :], qrot[hs, qsl],
                                    start=True, stop=True)
                            t2sb = phc.tile([12, L], f32r, tag="t2sb", bufs=2)
                            nc.scalar.copy(t2sb[:], gps[0:12, :])
                            gps2 = psC2.tile([64, L], f32, tag="geo_ps",
                                             bufs=1, name="b48ps")
                            for n in range(2):
                                nc.tensor.matmul(
                                    gps2[0:48, n * 512:(n + 1) * 512],
                                    rep_t[:], t2sb[:, n * 512:(n + 1) * 512],
                                    start=True, stop=True)
                            ab = phc.tile([48, L], f32r, tag="ab", bufs=2)
                            nc.vector.tensor_tensor(ab[:], gps2[0:48, :],
                                                    base_sb[b][:], OP.mult)
                            gps3 = psC2.tile([64, L], f32, tag="geo_ps",
                                             bufs=1, name="geops")
                            for n in range(2):
                                nc.tensor.matmul(
                                    gps3[:, n * 512:(n + 1) * 512],
                                    ew_t[:], ab[:, n * 512:(n + 1) * 512],
                                    start=True, stop=True)
                            if DEBUG and b == 0 and h == 0:
                                pxd = phc.tile([65, L], f32, tag="pxd")
                                nc.vector.tensor_copy(pxd[:], ctx_ps[0:65, :])
                                nc.sync.dma_start(d_dbg_att[0:65, :], pxd[:])
                            # division + combine (denominator must reach a
                            # base-0 row: partition_broadcast reads abs part 0)
                            dcp = phc.tile([65, L], f32, tag="recip", bufs=2)
                            nc.vector.tensor_copy(dcp[64:65, :],
                                                  ctx_ps[64:65, :])
                            den0 = phc.tile([1, L], f32, tag="den0", bufs=2)
                            nc.sync.dma_start(den0[:], dcp[64:65, :])
                            rec0 = phc.tile([1, L], f32, tag="rec0", bufs=2)
                            nc.vector.reciprocal(rec0[:], den0[:])
                            bcr = phc.tile([64, L], f32, tag="bcr", bufs=2)
                            nc.gpsimd.partition_broadcast(bcr[:], rec0[:],
                                                          channels=64)
                            fin1 = phc.tile([64, L], f32, tag="fin1", bufs=2)
                            nc.vector.tensor_tensor(fin1[:], ctx_ps[0:64, :],
                                                    bcr[:], OP.mult)
                            if DEBUG and b == 0 and h == 0:
                                nc.sync.dma_start(d_dbg_att[P:P + 64, :],
                                                  bcr[:])
                                nc.sync.dma_start(d_dbg_att[P + 64:P + 65, :],
                                                  rec0[:])
                                gpe = phc.tile([64, L], f32, tag="gpe")
                                nc.vector.tensor_copy(gpe[:], gps3[:])
                                nc.sync.dma_start(d_dbg_att[2 * P:2 * P + 64, :], gpe[:])
                            if h == 0:
                                nc.vector.tensor_tensor(ctxcat[b][0:64, :],
                                                        fin1[:], gps3[:],
                                                        OP.add)
                            else:
                                fin2 = phc.tile([64, L], f32r, tag="fin1",
                                                bufs=2, name="fin2")
                                nc.vector.tensor_tensor(fin2[:], fin1[:],
                                                        gps3[:], OP.add)
                                nc.sync.dma_start(ctxcat[b][64:128, :],
                                                  fin2[:])

                if DEBUG:
                    for b in range(B):
                        nc.sync.dma_start(d_dbg_ctx[b * P:(b + 1) * P, :],
                                          ctxcat[b][:].bitcast(f32))
                # out projection
                with tc.tile_pool(name="psC3", bufs=1, space="PSUM") as psC3:
                    for b in range(B):
                        for jo in range(NI):
                            for n in range(2):
                                po = psC3.tile([P, 512], f32, tag="out_ps",
                                               bufs=4)
                                nc.tensor.matmul(
                                    po[:], outw[:, jo * P:(jo + 1) * P],
                                    ctxcat[b][:, n * 512:(n + 1) * 512],
                                    start=True, stop=True)
                                ot = phc.tile([P, 512], f32, tag="ot", bufs=4)
                                if (jo + n) % 2 == 0:
                                    nc.vector.tensor_copy(ot[:], po[:])
                                else:
                                    nc.scalar.copy(ot[:], po[:])
                                nc.sync.dma_start(
                                    d_out[jo * P:(jo + 1) * P,
                                          b * L + n * 512:
                                          b * L + (n + 1) * 512],
                                    ot[:])
    nc.compile()
    return nc


def _ntff_shim():
    """Register antenv.axon_hooks with a ctypes NTFF profile hook."""
    import sys, types, ctypes, contextlib
    if "antenv.axon_hooks" in sys.modules:
        return
    lib = ctypes.CDLL("/opt/axon/libaxon_pjrt.so")
    lib.axon_start_nrt_profile.argtypes = [ctypes.POINTER(ctypes.c_int64),
                                           ctypes.c_size_t]
    lib.axon_start_nrt_profile.restype = ctypes.c_int64
    lib.axon_stop_nrt_profile.argtypes = [ctypes.c_char_p]
    lib.axon_stop_nrt_profile.restype = ctypes.c_int64

    @contextlib.contextmanager
    def _hook(output_dir, device_ids):
        import jax
        jax.devices()
        if device_ids:
            ids = (ctypes.c_int64 * len(device_ids))(*device_ids)
            rc = lib.axon_start_nrt_profile(ids, len(device_ids))
        else:
            rc = lib.axon_start_nrt_profile(None, 0)
        if rc != 0:
            raise RuntimeError(f"axon_start_nrt_profile rc={rc}")
        try:
            yield
        finally:
            n = lib.axon_stop_nrt_profile(str(output_dir).encode())
            print(f"profile: {n} file(s) written to {output_dir}")

    mod = types.ModuleType("antenv.axon_hooks")
    mod.get_axon_ntff_profile_hook = lambda: _hook
    mod.set_axon_ntff_profile_hook = lambda h: None
    sys.modules["antenv.axon_hooks"] = mod


def kernel(**inputs) -> np.ndarray:
    global LAST_EXEC_NS
    from concourse import bass_utils

    shared, per_core = _host_prep(inputs)
    if "nc" not in _CACHED:
        _CACHED["nc"] = _build_nc()
    nc = _CACHED["nc"]

    in_maps = []
    for c in range(N_CORES):
        pc = per_core[c]
        in_maps.append({
            "x_T": shared["x_T"], "w_own_T": pc["w_own_T"],
            "aug_mu": pc["aug_mu"], "aug_cb": pc["aug_cb"],
            "qlw_col": pc["qlw_col"],
            "klw_col": pc["klw_col"], "cos2": shared["cos2"],
            "sin2": shared["sin2"], "J2T": shared["J2T"],
            "outw_T": pc["outw_T"], "bm_sc_cm": shared["bm_sc_cm"],
            "base_ex": shared["base_ex"], "rep_lhsT": shared["rep_lhsT"],
            "ew_lhsT": shared["ew_lhsT"],
            "ones_over_D": shared["ones_over_D"],
            "ones_row": shared["ones_row"],
        })

    if TRACE:
        _ntff_shim()
    res = bass_utils.run_bass_kernel_spmd(
        nc, in_maps, core_ids=list(range(N_CORES)), trace=TRACE)
    LAST_EXEC_NS = res.exec_time_ns

    out_T = np.zeros((D, T), np.float64)
    for c in range(N_CORES):
        out_T += res.results[c]["out_T"].astype(np.float64)
    return np.ascontiguousarray(out_T.T.reshape(B, L, D).astype(np.float32))
